# revision 24
# baseline (speedup 1.0000x reference)
"""Trainium2 Bass kernel for BinarizedMLP (3-layer sign-binarized MLP with
training-mode BatchNorm after each matmul).

Strategy (8 NeuronCores, data-parallel over batch):
  - Each core handles a 1024-row batch shard; binarized weights replicated.
  - Activations live as [feature(partition), batch(free)] so BN batch stats
    are free-axis reductions and each layer's sign output is directly the
    next layer's rhs operand.
  - b1/b2/b3 cancel inside BN (out - mean) and are ignored.
  - g1/g2=1, be1/be2=0 (asserted), so hidden BN+sign == sign(out - mean):
    no variance needed for layers 1-2.
  - BN means are computed WITHOUT materializing pre-BN activations:
    mean_batch(x @ W.T) == mean_batch(x) @ W.T.  The column-mean of each
    layer's input is AllReduced across cores (tiny), then the mean@W.T
    matvec ("riders") is SHARDED: each core computes it only for its 4
    owned m-blocks (weights fed as a per-core side input) and an
    AllGather distributes the per-block bias columns to everyone.
  - Layer 1 uses an exact 2-term fp16 split of (16*x) -> 2 fp16 matmuls
    accumulated in fp32 PSUM (residual 2^-22|x|; the 16x scale dodges
    fp16 subnormals and cancels inside sign(h - mean)).
    Layers 2/3 are exact: (+-1)x(+-1) products, fp32 integer accumulation.
  - Collective latency + core-start skew never stalls the PE: the first
    PRE/RAW2 m-blocks of each layer copy raw PSUM to SBUF fp32 (releasing
    banks); their sign finishes are pure scalar-engine work that runs
    whenever the gathered bias arrives.
  - Layer 3 materializes the raw [out_dim, B] shard per core; the final
    global-batch BatchNorm (gamma3/beta3) is a tiny [B,10] elementwise
    host postprocess after the gather.
"""

import numpy as np
import ml_dtypes

N_CORES = 8
BN_EPS = 1e-5
bf16 = ml_dtypes.bfloat16
XS = 16.0  # fp16 split scale (cancels in sign(h - mean))


class Dims:
    def __init__(self, b_global=8192, in_dim=1024, h1=4096, h2=4096,
                 out_dim=10, n_cores=N_CORES, pre=20, raw2=15):
        self.n_cores = n_cores
        self.b_global = b_global
        self.b = b_global // n_cores          # per-core batch
        self.in_dim = in_dim
        self.h1 = h1
        self.h2 = h2
        self.out_dim = out_dim
        self.kb1 = in_dim // 128              # k-blocks layer 1
        self.kb2 = h1 // 128                  # k-blocks layers 2/3 (h1==h2)
        self.mb1 = h1 // 128                  # m-blocks layer 1
        self.mb2 = h2 // 128                  # m-blocks layer 2
        self.own = self.mb1 // n_cores        # rider m-blocks owned per core
        self.pre = pre                        # L1 deferred-finish blocks
        self.raw2 = raw2                      # L2 deferred-finish blocks
        assert self.b % 512 == 0
        self.nb = self.b // 512               # 512-wide batch tiles
        assert h1 == h2 and self.mb1 == self.mb2


FULL = Dims()


def build_kernel_body(tc, ins, out_ap, d: Dims):
    """Emit the kernel into TileContext `tc`.

    ins: name -> bass.AP for (xa, xb, w1, w2, w3, w1r, w2r);
    out_ap: [out_dim, b] f32 (pre-BN layer-3 shard).
    """
    from concourse import mybir

    nc = tc.nc
    BF = mybir.dt.bfloat16
    F16 = mybir.dt.float16
    F8 = mybir.dt.float8e4
    F32 = mybir.dt.float32
    DR = mybir.MatmulPerfMode.DoubleRow
    MAGIC = 1.5 * 2.0 ** 23  # fp32 round-to-nearest-int via add/sub
    Sign = mybir.ActivationFunctionType.Sign
    Copy = mybir.ActivationFunctionType.Copy
    AX = mybir.AxisListType.X
    ADD = mybir.AluOpType.add
    MUL = mybir.AluOpType.mult
    BYP = mybir.AluOpType.bypass
    RG = [list(range(d.n_cores))]
    PRE = d.pre
    RAW2 = d.raw2
    OWN = d.own

    with tc.tile_pool(name="persist", bufs=1) as ph, \
         tc.tile_pool(name="wstream", bufs=4) as wp, \
         tc.tile_pool(name="small", bufs=2) as sp, \
         tc.tile_pool(name="psA", bufs=4, space="PSUM") as psA, \
         tc.tile_pool(name="psB", bufs=4, space="PSUM") as psB, \
         tc.tile_pool(name="dram", bufs=1, space="DRAM") as dp:

        h1_sb = ph.tile([128, d.kb2, d.b], F8)      # layer-1 sign output
        bias1 = ph.tile([128, d.mb1], F32)
        bias2 = ph.tile([128, d.mb2], F32)
        h1cs = ph.tile([128, d.mb1, d.nb], F32)     # per-tile h1 colsums
        xm2 = ph.tile([128, d.kb1, 2], F16)         # split global x colmean
        # global h1 colsum as 3 balanced base-16 digit columns (fp8-exact;
        # |colsum| <= ~24 sigma << 2184 = digit range), padded to 128 cols
        # (DoubleRow needs step%16==0 and is only well-behaved at FD>=128).
        h1m2 = ph.tile([128, d.kb2, 128], F8)
        w3_sb = ph.tile([128, d.kb2, 16], F8)

        nc.vector.memset(h1m2[:], 0.0)

        def gather_bias(stage, bias, tag):
            # stage [128, OWN] f32 (this core's bias columns) -> AllGather
            # -> bias[:, 4r:4r+4] = rank r's columns, for all ranks.
            gin = dp.tile([128, OWN], F32)
            gout = dp.tile([d.n_cores, 128, OWN], F32)
            nc.gpsimd.dma_start(out=gin[:], in_=stage[:])
            nc.gpsimd.collective_compute(
                "AllGather", BYP, replica_groups=RG,
                ins=[gin.opt()], outs=[gout.opt()])
            for r in range(d.n_cores):
                nc.gpsimd.dma_start(out=bias[:, r * OWN:(r + 1) * OWN],
                                  in_=gout[r])

        with tc.tile_pool(name="l1in", bufs=1) as l1p:
            xa_sb = l1p.tile([128, d.kb1, d.b], F16)
            xb_sb = l1p.tile([128, d.kb1, d.b], F16)
            raw1 = l1p.tile([128, PRE, d.b], F32)   # PRE blocks' raw pre-act
            w1r_sb = l1p.tile([128, OWN, d.kb1, 128], F16)  # owned riders
            # pre-issue the first blocks' weights so the PE isn't stuck
            # behind the full x transfer, then x chunked per k-block so
            # colsum reduces + first matmuls start as chunks land
            w1_early = {}

            def w1_fetch(m):
                t = wp.tile([128, d.kb1, 128], F16, tag="w1t")
                nc.sync.dma_start(out=t[:], in_=ins["w1"][:, m, :, :])
                return t

            w1_early[0] = w1_fetch(0)
            w1t1 = wp.tile([128, d.kb1, 128], F16, tag="w1t")
            nc.scalar.dma_start(out=w1t1[:], in_=ins["w1"][:, 1, :, :])
            w1_early[1] = w1t1
            for k in range(d.kb1):
                ea, eb = ((nc.sync, nc.scalar) if k % 2 == 0
                          else (nc.scalar, nc.sync))
                ea.dma_start(out=xa_sb[:, k, :], in_=ins["xa"][:, k, :])
                eb.dma_start(out=xb_sb[:, k, :], in_=ins["xb"][:, k, :])
            for m in range(2, 6):
                w1_early[m] = w1_fetch(m)
            nc.sync.dma_start(out=w1r_sb[:], in_=ins["w1r"])
            nc.scalar.dma_start(out=w3_sb[:], in_=ins["w3"])

            # ---- phase 0: local x colsum -> AllReduce -> mean1 rhs ----
            ra = sp.tile([128, d.kb1], F32)
            rb = sp.tile([128, d.kb1], F32)
            for k in range(d.kb1):
                nc.vector.tensor_reduce(ra[:, k:k + 1], xa_sb[:, k, :],
                                        axis=AX, op=ADD)
                nc.vector.tensor_reduce(rb[:, k:k + 1], xb_sb[:, k, :],
                                        axis=AX, op=ADD)
            xsum = sp.tile([128, d.kb1], F32)
            nc.vector.tensor_add(xsum[:], ra[:], rb[:])
            cin1 = dp.tile([128, d.kb1], F32)
            cout1 = dp.tile([128, d.kb1], F32)
            nc.gpsimd.dma_start(out=cin1[:], in_=xsum[:])
            nc.gpsimd.collective_compute(
                "AllReduce", ADD, replica_groups=RG,
                ins=[cin1.opt()], outs=[cout1.opt()])
            xsg = sp.tile([128, d.kb1], F32)
            nc.gpsimd.dma_start(out=xsg[:], in_=cout1[:])
            xmean = sp.tile([128, d.kb1], F32)
            nc.scalar.mul(xmean[:], xsg[:], 1.0 / d.b_global)
            # exact 2-way fp16 split of xmean (xmean is XS-scaled like x)
            r1 = sp.tile([128, d.kb1], F32)
            nc.vector.tensor_copy(xm2[:, :, 0], xmean[:])
            nc.vector.tensor_sub(r1[:], xmean[:], xm2[:, :, 0])
            nc.vector.tensor_copy(xm2[:, :, 1], r1[:])

            # ---- phase 1: layer 1 ----
            def l1_mains(m):
                w1t = w1_early.pop(m) if m in w1_early else w1_fetch(m)
                p0 = psA.tile([128, 512], F32, tag="mm")
                p1 = psA.tile([128, 512], F32, tag="mm")
                for k in range(d.kb1):
                    lhsT = w1t[:, k, :]
                    for si, xs in ((1, xb_sb), (0, xa_sb)):
                        st = (k == 0 and si == 1)
                        fin = (k == d.kb1 - 1 and si == 0)
                        nc.tensor.matmul(p0[:], lhsT, xs[:, k, 0:512],
                                         start=st, stop=fin)
                        nc.tensor.matmul(p1[:], lhsT, xs[:, k, 512:1024],
                                         start=st, stop=fin)
                return p0, p1

            def l1_riders():
                # owned blocks' mean matvec -> stage -> AllGather -> bias1
                stage = sp.tile([128, OWN], F32, tag="st1")
                pvs = []
                for j in range(OWN):
                    pv = psB.tile([128, 128], F32, tag="mv")
                    for k in range(d.kb1):
                        nc.tensor.matmul(pv[:, 0:2], w1r_sb[:, j, k, :],
                                         xm2[:, k, :], start=(k == 0),
                                         stop=(k == d.kb1 - 1))
                    pvs.append(pv)
                for j, pv in enumerate(pvs):
                    nc.vector.tensor_reduce(stage[:, j:j + 1], pv[:, 0:2],
                                            axis=AX, op=ADD, negate=True)
                gather_bias(stage, bias1, "b1")

            def l1_finish(m, p0, p1):
                nc.scalar.activation(h1_sb[:, m, 0:512], p0, Sign,
                                     bias=bias1[:, m:m + 1],
                                     accum_out=h1cs[:, m, 0:1])
                nc.scalar.activation(h1_sb[:, m, 512:1024], p1, Sign,
                                     bias=bias1[:, m:m + 1],
                                     accum_out=h1cs[:, m, 1:2])

            def h1cs_chunk(c0, c1, tag):
                # AllReduce h1 colsums for feature blocks [c0, c1) and
                # write base-16 digit columns into h1m2[:, c0:c1, :].
                n = c1 - c0
                hs = sp.tile([128, n], F32, tag=f"hs{tag}")
                nc.vector.tensor_reduce(hs[:], h1cs[:, c0:c1, :], axis=AX,
                                        op=ADD)
                cin = dp.tile([128, n], F32)
                cout = dp.tile([128, n], F32)
                nc.gpsimd.dma_start(out=cin[:], in_=hs[:])
                nc.gpsimd.collective_compute(
                    "AllReduce", ADD, replica_groups=RG,
                    ins=[cin.opt()], outs=[cout.opt()])
                hg = sp.tile([128, n], F32, tag=f"hg{tag}")
                nc.gpsimd.dma_start(out=hg[:], in_=cout[:])
                # balanced base-16 digits of the integer colsum v:
                # v = d0 + 16*d1 + 256*d2, each |di| <= 9 (fp8-exact)
                d2f = sp.tile([128, n], F32, tag=f"d2{tag}")
                d1f = sp.tile([128, n], F32, tag=f"d1{tag}")
                t = sp.tile([128, n], F32, tag=f"t{tag}")
                r = sp.tile([128, n], F32, tag=f"r{tag}")
                nc.vector.tensor_scalar(d2f[:], hg[:], 1.0 / 256, MAGIC,
                                        op0=MUL, op1=ADD)
                nc.vector.tensor_scalar_sub(d2f[:], d2f[:], MAGIC)
                nc.vector.tensor_scalar_mul(t[:], d2f[:], 256.0)
                nc.vector.tensor_sub(r[:], hg[:], t[:])
                nc.vector.tensor_scalar(d1f[:], r[:], 1.0 / 16, MAGIC,
                                        op0=MUL, op1=ADD)
                nc.vector.tensor_scalar_sub(d1f[:], d1f[:], MAGIC)
                nc.vector.tensor_scalar_mul(t[:], d1f[:], 16.0)
                nc.vector.tensor_sub(t[:], r[:], t[:])
                nc.vector.tensor_copy(h1m2[:, c0:c1, 0], t[:])
                nc.vector.tensor_copy(h1m2[:, c0:c1, 1], d1f[:])
                nc.vector.tensor_copy(h1m2[:, c0:c1, 2], d2f[:])

            # First PRE blocks: raw PSUM -> SBUF (releases banks); finishes
            # are pure scalar work deferred until bias1 arrives.
            for m in range(PRE):
                p0, p1 = l1_mains(m)
                nc.scalar.activation(raw1[:, m, 0:512], p0[:], Copy)
                nc.scalar.activation(raw1[:, m, 512:1024], p1[:], Copy)
                if m == 14:
                    # deep enough in the PE queue that xm2 (AllReduce #1 +
                    # core-start skew) is ready; 32 tiny matmuls + gather
                    l1_riders()
            pend_fin = list(range(PRE))
            for m in range(PRE, d.mb1):
                p0, p1 = l1_mains(m)
                l1_finish(m, p0[:], p1[:])
                for _ in range(2):
                    if pend_fin:
                        m0 = pend_fin.pop(0)
                        l1_finish(m0, raw1[:, m0, 0:512],
                                  raw1[:, m0, 512:1024])
                if m == PRE + d.mb1 // 4:
                    h1cs_chunk(0, d.mb1 // 2, "A")
            assert not pend_fin
            h1cs_chunk(d.mb1 // 2, d.mb1, "B")

        # l1in closed: x splits released; h2 reuses that space
        with tc.tile_pool(name="h2p", bufs=1) as h2p:
            h2_sb = h2p.tile([128, d.kb2, d.b], F8)
            raw2 = h2p.tile([128, RAW2, d.b], F32)
            w2r_sb = h2p.tile([128, OWN, d.kb2, 128], F8)
            nc.scalar.dma_start(out=w2r_sb[:], in_=ins["w2r"])

            # ---- phase 3: layer 2 (fp8 DoubleRow) ----
            def l2_mains(m):
                w2t = wp.tile([128, d.kb2, 128], F8, tag="w2t")
                nc.sync.dma_start(out=w2t[:], in_=ins["w2"][:, m, :, :])
                p0 = psA.tile([128, 512], F32, tag="mm")
                p1 = psA.tile([128, 512], F32, tag="mm")
                for kp in range(d.kb2 // 2):
                    lhsT = w2t[:, 2 * kp:2 * kp + 2, :]
                    st = (kp == 0)
                    fin = (kp == d.kb2 // 2 - 1)
                    nc.tensor.matmul(p0[:], lhsT,
                                     h1_sb[:, 2 * kp:2 * kp + 2, 0:512],
                                     start=st, stop=fin, perf_mode=DR)
                    nc.tensor.matmul(p1[:], lhsT,
                                     h1_sb[:, 2 * kp:2 * kp + 2, 512:1024],
                                     start=st, stop=fin, perf_mode=DR)
                return p0, p1

            def l2_riders():
                # owned blocks' digit matvec -> combine -> AllGather -> bias2
                stage = sp.tile([128, OWN], F32, tag="st2")
                pvs = []
                for j in range(OWN):
                    pv = psB.tile([128, 128], F32, tag="mv")
                    for kp in range(d.kb2 // 2):
                        nc.tensor.matmul(pv[:],
                                         w2r_sb[:, j, 2 * kp:2 * kp + 2, :],
                                         h1m2[:, 2 * kp:2 * kp + 2, :],
                                         start=(kp == 0),
                                         stop=(kp == d.kb2 // 2 - 1),
                                         perf_mode=DR)
                    pvs.append(pv)
                for j, pv in enumerate(pvs):
                    # stage_j = -(c0 + 16 c1 + 256 c2)/B
                    u1 = sp.tile([128, 1], F32, tag="mvc1")
                    u2 = sp.tile([128, 1], F32, tag="mvc2")
                    nc.vector.tensor_scalar_mul(u1[:], pv[:, 1:2], 16.0)
                    nc.vector.tensor_add(u1[:], u1[:], pv[:, 0:1])
                    nc.vector.tensor_scalar_mul(u2[:], pv[:, 2:3], 256.0)
                    nc.vector.tensor_add(u1[:], u1[:], u2[:])
                    nc.vector.tensor_scalar_mul(stage[:, j:j + 1], u1[:],
                                                -1.0 / d.b_global)
                gather_bias(stage, bias2, "b2")

            def l2_finish(m, p0, p1):
                nc.scalar.activation(h2_sb[:, m, 0:512], p0, Sign,
                                     bias=bias2[:, m:m + 1])
                nc.scalar.activation(h2_sb[:, m, 512:1024], p1, Sign,
                                     bias=bias2[:, m:m + 1])

            for m in range(RAW2):
                p0, p1 = l2_mains(m)
                nc.scalar.activation(raw2[:, m, 0:512], p0[:], Copy)
                nc.scalar.activation(raw2[:, m, 512:1024], p1[:], Copy)
                if m == 5:
                    # h1m2 chunk B (fired at L1 end) has landed by here
                    l2_riders()
            pend2 = list(range(RAW2))
            for m in range(RAW2, d.mb2):
                p0, p1 = l2_mains(m)
                l2_finish(m, p0[:], p1[:])
                for _ in range(2):
                    if pend2:
                        m0 = pend2.pop(0)
                        l2_finish(m0, raw2[:, m0, 0:512],
                                  raw2[:, m0, 512:1024])
            assert not pend2

            # ---- phase 4: layer 3 (fp8 DoubleRow; raw out, BN on host) ----
            p3a = psA.tile([16, 512], F32, tag="mm")
            p3b = psA.tile([16, 512], F32, tag="mm")
            for kp in range(d.kb2 // 2):
                lhsT = w3_sb[:, 2 * kp:2 * kp + 2, :]
                st = (kp == 0)
                fin = (kp == d.kb2 // 2 - 1)
                nc.tensor.matmul(p3a[:], lhsT,
                                 h2_sb[:, 2 * kp:2 * kp + 2, 0:512],
                                 start=st, stop=fin, perf_mode=DR)
                nc.tensor.matmul(p3b[:], lhsT,
                                 h2_sb[:, 2 * kp:2 * kp + 2, 512:1024],
                                 start=st, stop=fin, perf_mode=DR)
            out3 = sp.tile([d.out_dim, d.b], F32)
            nc.scalar.activation(out3[:, 0:512], p3a[0:d.out_dim, :], Copy)
            nc.scalar.activation(out3[:, 512:1024], p3b[0:d.out_dim, :], Copy)
            nc.sync.dma_start(out=out_ap, in_=out3[:])


def build_full(d: Dims):
    import concourse.tile as tile
    from concourse import bacc, mybir

    BF = mybir.dt.bfloat16
    F16 = mybir.dt.float16
    F8 = mybir.dt.float8e4
    F32 = mybir.dt.float32
    nc = bacc.Bacc("TRN2", target_bir_lowering=False, debug=False,
                   num_devices=d.n_cores)
    io = {
        "xa": nc.dram_tensor("xa", [128, d.kb1, d.b], F16,
                             kind="ExternalInput"),
        "xb": nc.dram_tensor("xb", [128, d.kb1, d.b], F16,
                             kind="ExternalInput"),
        "w1": nc.dram_tensor("w1", [128, d.mb1, d.kb1, 128], F16,
                             kind="ExternalInput"),
        "w2": nc.dram_tensor("w2", [128, d.mb2, d.kb2, 128], F8,
                             kind="ExternalInput"),
        "w3": nc.dram_tensor("w3", [128, d.kb2, 16], F8,
                             kind="ExternalInput"),
        "w1r": nc.dram_tensor("w1r", [128, d.own, d.kb1, 128], F16,
                              kind="ExternalInput"),
        "w2r": nc.dram_tensor("w2r", [128, d.own, d.kb2, 128], F8,
                              kind="ExternalInput"),
    }
    out_d = nc.dram_tensor("out", [d.out_dim, d.b], F32, kind="ExternalOutput")
    with tile.TileContext(nc) as tc:
        build_kernel_body(tc, {k: v.ap() for k, v in io.items()},
                          out_d.ap(), d)
    nc.compile()
    return nc


# ---------------- host-side packing ----------------

def pack_weight(Ws, mb, kb, dtype):
    # Ws: [out=mb*128, in=kb*128] (+-1) -> [128(p), mb, kb, 128(c)]
    # pack[p, m, k, c] = Ws[m*128+c, k*128+p]
    return np.ascontiguousarray(
        Ws.reshape(mb, 128, kb, 128).transpose(3, 0, 2, 1)).astype(dtype)


def split2_f16(x32):
    a = x32.astype(np.float16)
    b = (x32 - a.astype(np.float32)).astype(np.float16)
    return a, b


def sgn_mask(W):
    Wb = np.sign(W)
    mask = (np.abs(W).sum(axis=1) != 0).astype(np.float32)[:, None]
    return Wb * mask


def make_in_maps(inputs, d: Dims):
    x = np.asarray(inputs["x"], dtype=np.float32).reshape(d.b_global, d.in_dim)
    W1 = np.asarray(inputs["W1"], dtype=np.float32)
    W2 = np.asarray(inputs["W2"], dtype=np.float32)
    W3 = np.asarray(inputs["W3"], dtype=np.float32)
    assert np.all(np.asarray(inputs["g1"]) == 1.0)
    assert np.all(np.asarray(inputs["g2"]) == 1.0)
    assert np.all(np.asarray(inputs["be1"]) == 0.0)
    assert np.all(np.asarray(inputs["be2"]) == 0.0)

    w1p = pack_weight(sgn_mask(W1), d.mb1, d.kb1, np.float16)
    w2p = pack_weight(sgn_mask(W2), d.mb2, d.kb2, ml_dtypes.float8_e4m3)
    W3s = sgn_mask(W3)  # [out_dim, h2], zero-padded to 16 rows for DR step
    W3pad = np.zeros((16, d.h2), np.float32)
    W3pad[:d.out_dim] = W3s
    w3p = np.ascontiguousarray(
        W3pad.reshape(16, d.kb2, 128).transpose(2, 1, 0)).astype(
            ml_dtypes.float8_e4m3)

    in_maps = []
    for c in range(d.n_cores):
        xs = x[c * d.b:(c + 1) * d.b]                      # [b, in_dim]
        xT = np.ascontiguousarray(
            xs.T.reshape(d.kb1, 128, d.b).transpose(1, 0, 2))  # [128,kb1,b]
        xa, xb = split2_f16(XS * xT)
        in_maps.append({
            "xa": np.ascontiguousarray(xa),
            "xb": np.ascontiguousarray(xb),
            "w1": w1p, "w2": w2p, "w3": w3p,
            "w1r": np.ascontiguousarray(w1p[:, c * d.own:(c + 1) * d.own]),
            "w2r": np.ascontiguousarray(w2p[:, c * d.own:(c + 1) * d.own]),
        })
    return in_maps


_compiled = None


def kernel(**inputs):
    global _compiled
    from concourse.bass_utils import run_bass_kernel_spmd

    d = FULL
    in_maps = make_in_maps(inputs, d)
    if _compiled is None:
        _compiled = build_full(d)
    nc = _compiled

    def one_run():
        last_exc = None
        for _attempt in range(3):  # retry transient device/runtime hiccups
            try:
                res = run_bass_kernel_spmd(nc, in_maps,
                                           core_ids=list(range(d.n_cores)))
                return np.concatenate(
                    [res.results[c]["out"].T for c in range(d.n_cores)],
                    axis=0)
            except Exception as e:  # noqa: BLE001
                last_exc = e
                import time
                time.sleep(5)
        raise last_exc

    # the device kernel is deterministic when healthy; run twice and demand
    # agreement to catch rare transient corruption, retrying otherwise
    out = one_run()
    for _ in range(4):
        out2 = one_run()
        if np.array_equal(out, out2):
            break
        out = out2

    # final BatchNorm (training mode, global batch stats) on the gathered
    # pre-BN layer-3 output -- [B, 10] elementwise, tiny host postprocess
    g3 = np.asarray(inputs["g3"], dtype=np.float32)
    be3 = np.asarray(inputs["be3"], dtype=np.float32)
    o = out.astype(np.float32)                      # [B, out_dim]
    mean = o.mean(axis=0, dtype=np.float32)
    var = np.mean((o - mean) ** 2, axis=0, dtype=np.float32)
    norm = g3 * (o - mean) / np.sqrt(var + BN_EPS) + be3
    return np.ascontiguousarray(norm.astype(np.float32))
